# revision 1
# baseline (speedup 1.0000x reference)
"""Trainium2 Bass kernel for nn_Net_SDE: 48-step neural SDE Monte-Carlo pricer.

Strategy: data-parallel over the 131072 MC samples across 8 NeuronCores
(16384 samples/core). Per core, per SDE step, four 3->128->128x3->1 MLPs are
evaluated for all samples with fp16 matmuls (fp32 PSUM accumulation), the
S/V state update runs in fp32 on the vector engine, and per-(maturity,strike)
payoff partial sums are reduced on-device; the host sums the 8 cores'
[128, 960] partials into the [96, 10] result.

Layouts (per core):
  - state tiles S,V: [128 partitions, 128 free], sample s = p*128 + f
  - MLP activations: [feature, sample-chunk] so hidden matmuls are K=128
  - sample rows for the first layer: inp [2, 16384] fp16 built per step by
    SBUF->SBUF flatten DMAs from centered fp16 copies of S, V
  - outputs of the 4 nets: col-tiled M=1 matmuls into PSUM partitions
    {0,32,64,96}, drained and scatter-DMA'd back to state layout
"""
import numpy as np
from contextlib import ExitStack

import orjson

import concourse.bass as bass
import concourse.tile as tile
from concourse import mybir

F16 = mybir.dt.float16
F32 = mybir.dt.float32
AF = mybir.ActivationFunctionType
OP = mybir.AluOpType

MC = 131072
N_STEPS = 48
N_CORES = 8
MCL = MC // N_CORES          # 16384 samples per core
N_GRP = MCL // 1024          # 16 groups of 1024 samples (2 psum-bank halves)

STRIKES_CALL = np.array([100., 105., 110., 115., 120., 125., 130., 135., 140., 145.], np.float32)
STRIKES_PUT = np.array([55., 60., 65., 70., 75., 80., 85., 90., 95., 100.], np.float32)


def _lap(t, off, dims):
    return bass.AP(tensor=t.tensor, offset=t.offset + off, ap=[list(d) for d in dims])


# ---------------------------------------------------------------------------
# Workaround: this walrus build accepts only ONE sync-wait command per
# instruction. Split any instruction with more waits into preceding
# same-engine Drain (ctrl no-op) instructions, one wait each — same-engine
# FIFO order makes this semantically identical.
def _split_sync_waits(bir_json: bytes) -> bytes:
    bir = orjson.loads(bir_json)
    for fn in bir.get("functions", []):
        for bb in fn.get("blocks", []):
            out = []
            changed = False
            for ins in bb.get("instructions", []):
                si = ins.get("sync_info") or {}
                waits = si.get("on_wait") or []
                if len(waits) > 1:
                    changed = True
                    for ci, w in enumerate(waits[:-1]):
                        out.append({
                            "name": f"{ins['name']}_sw{ci}",
                            "opcode": "Drain",
                            "engine": ins.get("engine", "SP"),
                            "ins": [], "outs": [],
                            "debug": ins.get("debug"),
                            "sync_info": {"on_update": [], "on_wait": [w]},
                        })
                    si["on_wait"] = waits[-1:]
                    ins["sync_info"] = si
                out.append(ins)
            if changed:
                bb["instructions"] = out
    return orjson.dumps(bir)


def _install_sync_split():
    import concourse.bass_utils as bu
    import concourse.bass2jax as b2j
    if getattr(bu, "_sync_split_installed", False):
        return
    orig = bu.compile_bir_kernel

    def patched(bir_json, tmpdir, neff_name="file.neff"):
        return orig(_split_sync_waits(bir_json), tmpdir, neff_name=neff_name)

    bu.compile_bir_kernel = patched
    bu._sync_split_installed = True
    if getattr(b2j, "compile_bir_kernel", None) is orig:
        b2j.compile_bir_kernel = patched


def build_nc(idx_steps, c0, bo0, bo1h, bo2, bo3, cS, cV, n_steps=N_STEPS,
             repeat=1):
    """Build the single-core Bass program (SPMD: all cores run the same code).

    idx_steps: list of 24 python ints (step index whose post-update S feeds
               output row i). c0 = 1 + r*h. bo* = output biases (net1 scaled by h).
    """
    nc = bass.Bass()

    z_in = nc.declare_dram_parameter("z", [n_steps, MCL], F32, isOutput=False)
    z1_in = nc.declare_dram_parameter("z1", [n_steps, MCL], F32, isOutput=False)
    wiT_in = nc.declare_dram_parameter("wiT", [2, 512], F16, isOutput=False)
    whT_in = nc.declare_dram_parameter("whT", [128, 1536], F16, isOutput=False)
    woT_in = nc.declare_dram_parameter("woT", [128, 4], F16, isOutput=False)
    b1_in = nc.declare_dram_parameter("b1", [n_steps, 512], F32, isOutput=False)
    bh_in = nc.declare_dram_parameter("bh", [128, 12], F32, isOutput=False)
    strk_in = nc.declare_dram_parameter("strk", [128, 40], F32, isOutput=False)
    acc_out = nc.declare_dram_parameter("acc", [128, 960], F32, isOutput=True)

    s_hist = nc.dram_tensor("s_hist", [n_steps, MCL], F32)

    with tile.TileContext(nc) as tc, ExitStack() as ctx:
        consts = ctx.enter_context(tc.tile_pool(name="consts", bufs=1))
        persist = ctx.enter_context(tc.tile_pool(name="persist", bufs=1))
        hpool = ctx.enter_context(tc.tile_pool(name="hpool", bufs=3))
        obpool = ctx.enter_context(tc.tile_pool(name="obpool", bufs=2))
        zpool = ctx.enter_context(tc.tile_pool(name="zpool", bufs=1))
        updpool = ctx.enter_context(tc.tile_pool(name="updpool", bufs=1))
        tailpool = ctx.enter_context(tc.tile_pool(name="tailpool", bufs=2))
        psmm = ctx.enter_context(tc.tile_pool(name="psmm", bufs=2, space="PSUM"))
        pspo = ctx.enter_context(tc.tile_pool(name="pspo", bufs=1, space="PSUM"))

        # constants
        wiT = consts.tile([2, 512], F16)
        nc.sync.dma_start(out=wiT, in_=wiT_in[:, :])
        whT = consts.tile([128, 1536], F16)
        nc.sync.dma_start(out=whT, in_=whT_in[:, :])
        woT = consts.tile([128, 4], F16)
        nc.sync.dma_start(out=woT, in_=woT_in[:, :])
        # b1 column for the current step is DMA'd per iteration (dynamic offset)
        bh = consts.tile([128, 12], F32)
        nc.sync.dma_start(out=bh, in_=bh_in[:, :])
        strk = consts.tile([128, 40], F32)   # bias cols: -CALL, +PUT, -PUT, +CALL
        nc.sync.dma_start(out=strk, in_=strk_in[:, :])

        # persistent state
        S = persist.tile([128, 128], F32)
        V = persist.tile([128, 128], F32)
        S16 = persist.tile([128, 128], F16)
        V16 = persist.tile([128, 128], F16)
        inp = persist.tile([2, MCL], F16)
        outs_sq = persist.tile([128, 512], F32)
        acc = persist.tile([128, 960], F32)
        poA = pspo.tile([128, 1024], F32, tag="poA")
        poB = pspo.tile([128, 1024], F32, tag="poB")

        nc.vector.memset(S[:, :], cS)      # S starts at S0 (== centering constant)
        nc.vector.memset(V[:, :], cV)
        nc.vector.memset(S16[:, :], 0.0)   # centered: S0 - cS = 0
        nc.vector.memset(V16[:, :], 0.0)
        nc.vector.memset(outs_sq[:, :], 0.0)
        nc.vector.memset(poA[:, :], 0.0)
        nc.vector.memset(poB[:, :], 0.0)

        upd_a = updpool.tile([128, 128], F32, tag="upd_a")
        upd_b = updpool.tile([128, 128], F32, tag="upd_b")
        upd_c = updpool.tile([128, 128], F32, tag="upd_c")

        # ---- main SDE loop (repeat>1 is a timing-only mode) ----
        rep_ctx = (tc.For_i(0, repeat, 1) if repeat > 1 else None)
        if rep_ctx is not None:
            rep_ctx.__enter__()
        with tc.For_i(0, n_steps, 1,
                      hint_engines=(mybir.EngineType.PE, mybir.EngineType.Activation,
                                    mybir.EngineType.DVE, mybir.EngineType.SP)) as t:
            # first-layer input rows (from previous step's centered state);
            # both sides linearize partition-major, giving the p*128+f flatten
            nc.sync.dma_start(out=inp[0:1, :], in_=S16[:, :])
            nc.sync.dma_start(out=inp[1:2, :], in_=V16[:, :])
            # brownian increments for this step (prescaled by sqrt(h) on host)
            z_t = zpool.tile([128, 128], F32, tag="z")
            nc.sync.dma_start(out=z_t[:, :], in_=z_in[bass.ds(t, 1), :])
            z1_t = zpool.tile([128, 128], F32, tag="z1")
            nc.sync.dma_start(out=z1_t[:, :], in_=z1_in[bass.ds(t, 1), :])
            # first-layer bias for this step: [128, 4] (partition=feature, col=net)
            b1step = zpool.tile([128, 4], F32, tag="b1step")
            nc.sync.dma_start(out=b1step[:, :], in_=b1_in[bass.ds(t, 1), :])

            for g in range(N_GRP):
                po = poA if g % 2 == 0 else poB
                for n in range(4):
                    pm = psmm.tile([128, 1024], F32, tag="mm")
                    for hlf in range(2):
                        sl = slice(hlf * 512, hlf * 512 + 512)
                        isl = slice(g * 1024 + hlf * 512, g * 1024 + hlf * 512 + 512)
                        nc.tensor.matmul(pm[:, sl], lhsT=wiT[:, n * 128:(n + 1) * 128],
                                         rhs=inp[:, isl], start=True, stop=True)
                    h = hpool.tile([128, 1024], F16, tag="h")
                    bias1 = b1step[:, n:n + 1]
                    if n < 2:
                        nc.scalar.activation(h, pm[:, :], AF.Relu, bias=bias1, scale=1.0)
                    else:
                        nc.vector.tensor_scalar(out=h, in0=pm[:, :], scalar1=bias1,
                                                scalar2=0.0, op0=OP.add, op1=OP.max)
                    for l in range(3):
                        pm2 = psmm.tile([128, 1024], F32, tag="mm")
                        w_sl = slice((n * 3 + l) * 128, (n * 3 + l + 1) * 128)
                        for hlf in range(2):
                            sl = slice(hlf * 512, hlf * 512 + 512)
                            nc.tensor.matmul(pm2[:, sl], lhsT=whT[:, w_sl],
                                             rhs=h[:, sl], start=True, stop=True)
                        h2 = hpool.tile([128, 1024], F16, tag="h")
                        biasl = bh[:, n * 3 + l:n * 3 + l + 1]
                        # ACT handles nets 0,1 and (2,l=0); DVE the rest
                        if n < 2 or (n == 2 and l == 0):
                            nc.scalar.activation(h2, pm2[:, :], AF.Relu, bias=biasl, scale=1.0)
                        else:
                            nc.vector.tensor_scalar(out=h2, in0=pm2[:, :], scalar1=biasl,
                                                    scalar2=0.0, op0=OP.add, op1=OP.max)
                        h = h2
                    for hlf in range(2):
                        sl = slice(hlf * 512, hlf * 512 + 512)
                        nc.tensor.matmul(po[32 * n:32 * n + 1, sl], lhsT=woT[:, n:n + 1],
                                         rhs=h[:, sl], start=True, stop=True,
                                         tile_position=(0, 32 * n))
                # drain + scatter the 4 nets' outputs back to state layout
                ob = obpool.tile([128, 1024], F32, tag="ob")
                nc.vector.tensor_copy(ob, po[:, :])
                for n in range(4):
                    nc.sync.dma_start(
                        out=outs_sq[8 * g:8 * g + 8, 128 * n:128 * n + 128],
                        in_=ob[32 * n:32 * n + 1, :])

            # ---- state update (fp32, DVE) ----
            diff_r = outs_sq[:, 0:128]
            driftV_r = outs_sq[:, 128:256]
            diffV_r = outs_sq[:, 256:384]
            diffV1_r = outs_sq[:, 384:512]
            # S_new = relu(c0*S + (diff+bo0)*dW)
            nc.vector.scalar_tensor_tensor(out=upd_a, in0=diff_r, scalar=bo0,
                                           in1=z_t, op0=OP.add, op1=OP.mult)
            nc.vector.scalar_tensor_tensor(out=upd_b, in0=S, scalar=c0,
                                           in1=upd_a, op0=OP.mult, op1=OP.add)
            nc.vector.tensor_scalar(out=S, in0=upd_b, scalar1=0.0, scalar2=None,
                                    op0=OP.max)
            # V_new = V + (driftV*h + bo1*h) + (diffV+bo2)*dW + (diffV1+bo3)*dW1
            nc.vector.scalar_tensor_tensor(out=upd_a, in0=driftV_r, scalar=bo1h,
                                           in1=V, op0=OP.add, op1=OP.add)
            nc.vector.scalar_tensor_tensor(out=upd_b, in0=diffV_r, scalar=bo2,
                                           in1=z_t, op0=OP.add, op1=OP.mult)
            nc.vector.scalar_tensor_tensor(out=upd_c, in0=diffV1_r, scalar=bo3,
                                           in1=z1_t, op0=OP.add, op1=OP.mult)
            nc.vector.tensor_tensor(out=V, in0=upd_a, in1=upd_b, op=OP.add)
            nc.vector.tensor_tensor(out=V, in0=V, in1=upd_c, op=OP.add)
            # centered fp16 copies for next step's first layer
            nc.vector.tensor_scalar(out=S16, in0=S, scalar1=cS, scalar2=None,
                                    op0=OP.subtract)
            nc.vector.tensor_scalar(out=V16, in0=V, scalar1=cV, scalar2=None,
                                    op0=OP.subtract)
            # save S trajectory for the payoff phase
            nc.sync.dma_start(out=s_hist[bass.ds(t, 1), :], in_=S[:, :])
        if rep_ctx is not None:
            rep_ctx.__exit__(None, None, None)

        # ---- payoff phase (indices baked at trace time) ----
        # acc column layout per maturity i: i*40 + [0:10]=relu(S-Kc),
        #   [10:20]=relu(Kp-S), [20:30]=relu(S-Kp), [30:40]=relu(Kc-S)
        # All via ACT: out = relu(scale*S + bias), accum_out = per-partition sum.
        for i, step in enumerate(idx_steps):
            sh = tailpool.tile([128, 128], F32, tag="sh")
            nc.sync.dma_start(out=sh[:, :], in_=s_hist[step:step + 1, :])
            for j in range(40):
                scale = 1.0 if (j < 10 or 20 <= j < 30) else -1.0
                col = acc[:, i * 40 + j: i * 40 + j + 1]
                junk = tailpool.tile([128, 128], F32, tag="junk")
                nc.scalar.activation(junk, sh, AF.Relu,
                                     bias=strk[:, j:j + 1], scale=scale,
                                     accum_out=col)
        nc.sync.dma_start(out=acc_out[:, :], in_=acc)

    return nc


def _prep_inputs(S0, V0, rate, z, z1, indices, timegrid, Wi, bi, Wh, bh, Wo, bo,
                 n_steps=N_STEPS):
    """Host-side preprocessing. Returns (nc build args, per-core input maps, disc, idx)."""
    S0v = float(np.asarray(S0).reshape(-1)[0])
    V0v = float(np.asarray(V0).reshape(-1)[0])
    r = float(np.asarray(rate).reshape(-1)[0])
    tg = np.asarray(timegrid, np.float64)
    h = float(tg[1] - tg[0])
    sqh = float(np.sqrt(h))
    c0 = 1.0 + r * h

    Wi = np.asarray(Wi, np.float32)
    bi = np.asarray(bi, np.float32)
    Wh = np.asarray(Wh, np.float32)
    bhv = np.asarray(bh, np.float32)
    Wo = np.asarray(Wo, np.float32).copy()
    bo = np.asarray(bo, np.float32).copy()
    # driftV net (index 1) is only ever used multiplied by h -> fold h into it
    Wo[1] *= h
    bo0, bo1h, bo2, bo3 = float(bo[0, 0]), float(bo[1, 0]) * h, float(bo[2, 0]), float(bo[3, 0])

    cS, cV = S0v, V0v    # centering constants for fp16 inputs
    # first-layer bias with t-term and centering folded in: [4, n_steps, 128]
    t_vals = tg[:n_steps].astype(np.float32)
    b1 = (bi[:, None, :] + t_vals[None, :, None] * Wi[:, 0][:, None, :]
          + cS * Wi[:, 1][:, None, :] + cV * Wi[:, 2][:, None, :])   # [4, T, 128]
    # device layout: [n_steps, 512] with row t ordered (feature p, net n)
    b1_dev = np.ascontiguousarray(b1.transpose(1, 2, 0).reshape(n_steps, 512), np.float32)

    wiT_dev = np.ascontiguousarray(
        Wi[:, 1:3, :].transpose(1, 0, 2).reshape(2, 4 * 128), np.float16)
    whT_dev = np.ascontiguousarray(
        Wh.transpose(2, 0, 1, 3).reshape(128, 12 * 128), np.float16)
    woT_dev = np.ascontiguousarray(Wo[:, :, 0].T, np.float16)
    bh_dev = np.ascontiguousarray(bhv.transpose(2, 0, 1).reshape(128, 12), np.float32)

    strk_dev = np.ascontiguousarray(
        np.tile(np.concatenate([-STRIKES_CALL, STRIKES_PUT,
                                -STRIKES_PUT, STRIKES_CALL])[None, :], (128, 1)),
        np.float32)

    idx = np.asarray(indices).astype(np.int64).reshape(-1)
    idx_steps = [int((v - 1) % n_steps) for v in idx]
    disc = np.exp(-r * 2.0 * idx.astype(np.float64) / n_steps).astype(np.float64)

    z = np.asarray(z, np.float32)
    z1 = np.asarray(z1, np.float32)
    in_maps = []
    for k in range(N_CORES):
        sl = slice(k * MCL, (k + 1) * MCL)
        in_maps.append({
            "z": np.ascontiguousarray((z[sl, :n_steps] * sqh).T, np.float32),
            "z1": np.ascontiguousarray((z1[sl, :n_steps] * sqh).T, np.float32),
            "wiT": wiT_dev, "whT": whT_dev, "woT": woT_dev,
            "b1": b1_dev, "bh": bh_dev, "strk": strk_dev,
        })
    build_args = dict(idx_steps=idx_steps, c0=c0, bo0=bo0, bo1h=bo1h,
                      bo2=bo2, bo3=bo3, cS=cS, cV=cV, n_steps=n_steps)
    return build_args, in_maps, disc, idx_steps, (S0v, V0v, cS, cV)


def _combine(results, disc):
    """Sum per-core [128, 960] partials into the [96, 10] output."""
    total = np.zeros((128, 960), np.float64)
    for res in results:
        total += np.asarray(res["acc"], np.float64)
    cols = total.sum(axis=0).reshape(24, 40)
    calls_c = cols[:, 0:10]
    puts_p = cols[:, 10:20]
    calls_p = cols[:, 20:30]
    puts_c = cols[:, 30:40]
    out = np.concatenate([calls_c, puts_p, calls_p, puts_c], axis=0) / MC
    out = out * np.concatenate([disc] * 4)[:, None]
    return out.astype(np.float32)


def kernel(**inputs) -> np.ndarray:
    from concourse.bass_utils import run_bass_kernel_spmd
    _install_sync_split()
    build_args, in_maps, disc, _, _ = _prep_inputs(**inputs)
    nc = build_nc(**build_args)
    res = run_bass_kernel_spmd(nc, in_maps, list(range(N_CORES)))
    return _combine(res.results, disc)



# revision 7
# speedup vs baseline: 2.7470x; 2.7470x over previous
"""Trainium2 Bass kernel for nn_Net_SDE: 48-step neural SDE Monte-Carlo pricer.

Data-parallel over the 131072 MC samples across 8 NeuronCores (16384/core).

Per core the 48 SDE steps are fully unrolled (no Tile loop back-edge barriers)
and each step is processed in four 4096-sample "quarters" so that the
inter-step boundary work (state update, layout shuffles) of quarter q hides
under the MLP streams of quarter q+1.

Within a quarter, the 4 MLPs are evaluated layer-wise as weight-stationary
streams of 8 N=512 fp16 matmuls into two [128, 2048] (4-bank) PSUM tiles;
each PSUM tile is drained by a single FD=2048 bias+relu op that alternates
between the vector (DVE) and scalar (ACT) engines — the two drain engines
together are the throughput limit of the whole kernel.  The first layer is
row-tiled (tile_position=(32n,0), K=2) and the output layer is col-tiled
(tile_position=(0,32n), M=1) so the four nets share the PE array.

Payoffs use put-call parity: only call partials relu(S-K) and sum(S) are
computed on-device (21 ops per maturity instead of 40); the host derives the
put columns exactly as relu(K-S) = relu(S-K) - (S-K).
"""
import numpy as np
from contextlib import ExitStack

import orjson

import concourse.bass as bass
import concourse.tile as tile
from concourse import mybir

F16 = mybir.dt.float16
F32 = mybir.dt.float32
AF = mybir.ActivationFunctionType
OP = mybir.AluOpType

MC = 131072
N_STEPS = 48
N_CORES = 8
MCL = MC // N_CORES          # 16384 samples per core
GF = 128                     # state grid: [128 partitions, GF free]
NQ = 4                       # quarters per step
QF = GF // NQ                # grid cols per quarter (32 -> 4096 samples)
QS = 128 * QF                # samples per quarter
CH = 512                     # matmul moving-dim chunk
PMW = 2048                   # psum tile free width (4 banks)

STRIKES_CALL = np.array([100., 105., 110., 115., 120., 125., 130., 135., 140., 145.], np.float32)
STRIKES_PUT = np.array([55., 60., 65., 70., 75., 80., 85., 90., 95., 100.], np.float32)


# ---------------------------------------------------------------------------
# Workaround: this walrus build accepts only ONE sync-wait command per
# instruction. Split any instruction with more waits into preceding
# same-engine Drain (ctrl no-op) instructions, one wait each — same-engine
# FIFO order makes this semantically identical.
def _split_sync_waits(bir_json: bytes) -> bytes:
    bir = orjson.loads(bir_json)
    for fn in bir.get("functions", []):
        for bb in fn.get("blocks", []):
            out = []
            changed = False
            for ins in bb.get("instructions", []):
                si = ins.get("sync_info") or {}
                waits = si.get("on_wait") or []
                if len(waits) > 1:
                    changed = True
                    for ci, w in enumerate(waits[:-1]):
                        out.append({
                            "name": f"{ins['name']}_sw{ci}",
                            "opcode": "Drain",
                            "engine": ins.get("engine", "SP"),
                            "ins": [], "outs": [],
                            "debug": ins.get("debug"),
                            "sync_info": {"on_update": [], "on_wait": [w]},
                        })
                    si["on_wait"] = waits[-1:]
                    ins["sync_info"] = si
                out.append(ins)
            if changed:
                bb["instructions"] = out
    return orjson.dumps(bir)


def _install_sync_split():
    import concourse.bass_utils as bu
    import concourse.bass2jax as b2j
    if getattr(bu, "_sync_split_installed", False):
        return
    orig = bu.compile_bir_kernel

    def patched(bir_json, tmpdir, neff_name="file.neff"):
        return orig(_split_sync_waits(bir_json), tmpdir, neff_name=neff_name)

    bu.compile_bir_kernel = patched
    bu._sync_split_installed = True
    if getattr(b2j, "compile_bir_kernel", None) is orig:
        b2j.compile_bir_kernel = patched


def build_nc(idx_steps, c0, bo0, bo1h, bo2, bo3, cS, cV, n_steps=N_STEPS,
             repeat=1):
    """Build the single-core Bass program (SPMD: all cores run the same code).

    idx_steps: list of 24 ints (step whose post-update S feeds output row i).
    c0 = 1 + r*h. bo* = output biases (net 1 pre-scaled by h).
    """
    nc = bass.Bass()

    z_in = nc.declare_dram_parameter("z", [n_steps, MCL], F32, isOutput=False)
    z1_in = nc.declare_dram_parameter("z1", [n_steps, MCL], F32, isOutput=False)
    wiT_in = nc.declare_dram_parameter("wiT", [128, 128], F16, isOutput=False)
    whT_in = nc.declare_dram_parameter("whT", [128, 1536], F16, isOutput=False)
    woT_in = nc.declare_dram_parameter("woT", [128, 16], F16, isOutput=False)
    b1_in = nc.declare_dram_parameter("b1", [128, 4 * n_steps], F32, isOutput=False)
    bh_in = nc.declare_dram_parameter("bh", [128, 12], F32, isOutput=False)
    strk_in = nc.declare_dram_parameter("strk", [128, 20], F32, isOutput=False)
    n_mat = len(idx_steps)
    acc_out = nc.declare_dram_parameter("acc", [128, 21 * n_mat], F32, isOutput=True)

    # maturity -> list of output rows (handles duplicate indices)
    mat_map = {}
    for i, st in enumerate(idx_steps):
        mat_map.setdefault(st, []).append(i)

    with tile.TileContext(nc) as tc, ExitStack() as ctx:
        consts = ctx.enter_context(tc.tile_pool(name="consts", bufs=1))
        persist = ctx.enter_context(tc.tile_pool(name="persist", bufs=1))
        hpool = ctx.enter_context(tc.tile_pool(name="hpool", bufs=2))
        inppool = ctx.enter_context(tc.tile_pool(name="inppool", bufs=5))
        orowpool = ctx.enter_context(tc.tile_pool(name="orowpool", bufs=2))
        outspool = ctx.enter_context(tc.tile_pool(name="outspool", bufs=2))
        zpool = ctx.enter_context(tc.tile_pool(name="zpool", bufs=2))
        updpool = ctx.enter_context(tc.tile_pool(name="updpool", bufs=1))
        junkpool = ctx.enter_context(tc.tile_pool(name="junkpool", bufs=2))
        psmm = ctx.enter_context(tc.tile_pool(name="psmm", bufs=2, space="PSUM"))

        # ---- constants ----
        wiT = consts.tile([128, 128], F16)
        nc.sync.dma_start(out=wiT, in_=wiT_in[:, :])
        whT = consts.tile([128, 1536], F16)
        nc.sync.dma_start(out=whT, in_=whT_in[:, :])
        woT = consts.tile([128, 16], F16)
        nc.sync.dma_start(out=woT, in_=woT_in[:, :])
        b1 = consts.tile([128, 4 * n_steps], F32)
        nc.sync.dma_start(out=b1, in_=b1_in[:, :])
        bh = consts.tile([128, 12], F32)
        nc.sync.dma_start(out=bh, in_=bh_in[:, :])
        strk = consts.tile([128, 20], F32)
        nc.sync.dma_start(out=strk, in_=strk_in[:, :])

        # ---- persistent state ----
        S = persist.tile([128, GF], F32)
        V = persist.tile([128, GF], F32)
        S16 = persist.tile([128, GF], F16)
        V16 = persist.tile([128, GF], F16)
        acc = persist.tile([128, 21 * n_mat], F32)

        nc.vector.memset(S[:, :], cS)
        nc.vector.memset(V[:, :], cV)
        nc.vector.memset(S16[:, :], 0.0)
        nc.vector.memset(V16[:, :], 0.0)

        ua = updpool.tile([128, QF], F32, tag="ua")
        ub = updpool.tile([128, QF], F32, tag="ub")
        uc = updpool.tile([128, QF], F32, tag="uc")
        ud = updpool.tile([128, QF], F32, tag="ud")

        # drain engine alternation: DVE is slightly slower per drain, so it
        # takes 9 of every 20 big drains; payoff/small ops alternate evenly.
        state = {"k": 0, "pj": 0}

        def drain_relu(dst, pm_ap, bias_ap):
            k = state["k"]
            state["k"] += 1
            if (k * 9) // 20 != ((k + 1) * 9) // 20:
                nc.vector.tensor_scalar(out=dst, in0=pm_ap, scalar1=bias_ap,
                                        scalar2=0.0, op0=OP.add, op1=OP.max)
            else:
                nc.scalar.activation(dst, pm_ap, AF.Relu, bias=bias_ap, scale=1.0)

        def drain_copy(dst, pm_ap):
            k = state["k"]
            state["k"] += 1
            if (k * 9) // 20 != ((k + 1) * 9) // 20:
                nc.vector.tensor_copy(dst, pm_ap)
            else:
                nc.scalar.copy(dst, pm_ap)

        # initial first-layer input: centered state is exactly zero
        inp_tiles = []
        for q in range(NQ):
            t0 = inppool.tile([128, QS], F16, tag="inp")
            nc.vector.memset(t0[:, :], 0.0)
            inp_tiles.append(t0)

        def emit_payoff(row):
            # ACT only: DVE's accum_out does not produce free-dim sums on
            # this stack (verified empirically — it returns the last element).
            base = 21 * row
            for j in range(20):
                junk = junkpool.tile([128, GF], F32, tag="junk")
                col = acc[:, base + j:base + j + 1]
                nc.scalar.activation(junk, S, AF.Relu, bias=strk[:, j:j + 1],
                                     scale=1.0, accum_out=col)
            junk = junkpool.tile([128, GF], F32, tag="junk")
            nc.scalar.activation(junk, S, AF.Copy,
                                 accum_out=acc[:, base + 20:base + 21])

        # ---- main SDE loop (fully unrolled; repeat>1 is timing-only) ----
        rep_ctx = (tc.For_i(0, repeat, 1) if repeat > 1 else None)
        if rep_ctx is not None:
            rep_ctx.__enter__()
        for t in range(n_steps):
            z_t = zpool.tile([128, GF], F32, tag="z")
            nc.sync.dma_start(out=z_t[:, :], in_=z_in[t:t + 1, :])
            z1_t = zpool.tile([128, GF], F32, tag="z1")
            nc.sync.dma_start(out=z1_t[:, :], in_=z1_in[t:t + 1, :])

            for q in range(NQ):
                qs = slice(QF * q, QF * q + QF)
                inp_q = inp_tiles[q]
                # -- first layer: row-tiled K=2, one stream per net --
                hcur = []
                for n in range(4):
                    hn = hpool.tile([128, QS], F16, tag=f"h{n}")
                    for half in range(2):
                        pm = psmm.tile([128, PMW], F32, tag="pm")
                        for c in range(4):
                            col = half * PMW + c * CH
                            nc.tensor.matmul(
                                pm[:, c * CH:(c + 1) * CH],
                                lhsT=wiT[32 * n:32 * n + 2, :],
                                rhs=inp_q[32 * n:32 * n + 2, col:col + CH],
                                start=True, stop=True, tile_position=(32 * n, 0))
                        drain_relu(hn[:, half * PMW:(half + 1) * PMW], pm[:, :],
                                   b1[:, 4 * t + n:4 * t + n + 1])
                    hcur.append(hn)
                # -- hidden layers --
                for l in range(3):
                    for n in range(4):
                        hn = hpool.tile([128, QS], F16, tag=f"h{n}")
                        w_sl = slice((n * 3 + l) * 128, (n * 3 + l + 1) * 128)
                        for half in range(2):
                            pm = psmm.tile([128, PMW], F32, tag="pm")
                            for c in range(4):
                                col = half * PMW + c * CH
                                nc.tensor.matmul(
                                    pm[:, c * CH:(c + 1) * CH],
                                    lhsT=whT[:, w_sl],
                                    rhs=hcur[n][:, col:col + CH],
                                    start=True, stop=True)
                            drain_relu(hn[:, half * PMW:(half + 1) * PMW], pm[:, :],
                                       bh[:, n * 3 + l:n * 3 + l + 1])
                        hcur[n] = hn
                # -- output layer: M=4 zero-padded weights, PSUM-accumulated
                # across the 4 nets so outputs land on partitions 0-3 --
                orow = orowpool.tile([4, QS], F32, tag="orow")
                for half in range(2):
                    pmo = psmm.tile([128, PMW], F32, tag="pm")
                    for c in range(4):
                        for n in range(4):
                            col = half * PMW + c * CH
                            nc.tensor.matmul(
                                pmo[0:4, c * CH:(c + 1) * CH],
                                lhsT=woT[:, 4 * n:4 * n + 4],
                                rhs=hcur[n][:, col:col + CH],
                                start=(n == 0), stop=(n == 3))
                    drain_copy(orow[0:4, half * PMW:(half + 1) * PMW],
                               pmo[0:4, :])
                # -- scatter net outputs back to the state grid --
                og = []
                for n in range(4):
                    g = outspool.tile([128, QF], F32, tag=f"og{n}")
                    nc.sync.dma_start(out=g[:, :], in_=orow[n:n + 1, :])
                    og.append(g)
                # -- state update (fp32, DVE) --
                Ssl = S[:, qs]
                Vsl = V[:, qs]
                zsl = z_t[:, qs]
                z1sl = z1_t[:, qs]
                # S' = relu(c0*S + (diff+bo0)*dW)
                nc.vector.scalar_tensor_tensor(out=ua, in0=og[0], scalar=bo0,
                                               in1=zsl, op0=OP.add, op1=OP.mult)
                nc.vector.scalar_tensor_tensor(out=ub, in0=Ssl, scalar=c0,
                                               in1=ua, op0=OP.mult, op1=OP.add)
                nc.vector.tensor_scalar(out=Ssl, in0=ub, scalar1=0.0, scalar2=None,
                                        op0=OP.max)
                # V' = V + (driftV*h+bo1h) + (diffV+bo2)*dW + (diffV1+bo3)*dW1
                nc.vector.scalar_tensor_tensor(out=ua, in0=og[1], scalar=bo1h,
                                               in1=Vsl, op0=OP.add, op1=OP.add)
                nc.vector.scalar_tensor_tensor(out=ub, in0=og[2], scalar=bo2,
                                               in1=zsl, op0=OP.add, op1=OP.mult)
                nc.vector.scalar_tensor_tensor(out=uc, in0=og[3], scalar=bo3,
                                               in1=z1sl, op0=OP.add, op1=OP.mult)
                nc.vector.tensor_tensor(out=ud, in0=ua, in1=ub, op=OP.add)
                nc.vector.tensor_tensor(out=Vsl, in0=ud, in1=uc, op=OP.add)
                # centered fp16 copies for the next step's first layer
                nc.vector.tensor_scalar(out=S16[:, qs], in0=Ssl, scalar1=cS,
                                        scalar2=None, op0=OP.subtract)
                nc.vector.tensor_scalar(out=V16[:, qs], in0=Vsl, scalar1=cV,
                                        scalar2=None, op0=OP.subtract)
                # build next step's first-layer input rows (4 replicas for
                # the row-tiled first layer)
                if t + 1 < n_steps or repeat > 1:
                    ninp = inppool.tile([128, QS], F16, tag="inp")
                    for n in range(4):
                        nc.sync.dma_start(out=ninp[32 * n:32 * n + 1, :],
                                          in_=S16[:, qs])
                        nc.sync.dma_start(out=ninp[32 * n + 1:32 * n + 2, :],
                                          in_=V16[:, qs])
                    inp_tiles[q] = ninp

            # payoffs for maturities at this step (post-update S)
            for row in mat_map.get(t, []):
                emit_payoff(row)
        if rep_ctx is not None:
            rep_ctx.__exit__(None, None, None)

        nc.sync.dma_start(out=acc_out[:, :], in_=acc)

    return nc


def _prep_inputs(S0, V0, rate, z, z1, indices, timegrid, Wi, bi, Wh, bh, Wo, bo,
                 n_steps=None):
    """Host-side preprocessing. Returns (build args, per-core inputs, disc)."""
    S0v = float(np.asarray(S0).reshape(-1)[0])
    V0v = float(np.asarray(V0).reshape(-1)[0])
    r = float(np.asarray(rate).reshape(-1)[0])
    z = np.asarray(z, np.float32)
    z1 = np.asarray(z1, np.float32)
    if n_steps is None:
        n_steps = z.shape[1]
    tg = np.asarray(timegrid, np.float64)
    h = float(tg[1] - tg[0])
    sqh = float(np.sqrt(h))
    c0 = 1.0 + r * h

    Wi = np.asarray(Wi, np.float32)
    bi = np.asarray(bi, np.float32)
    Wh = np.asarray(Wh, np.float32)
    bhv = np.asarray(bh, np.float32)
    Wo = np.asarray(Wo, np.float32).copy()
    bo = np.asarray(bo, np.float32).copy()
    # driftV net (index 1) is only ever used multiplied by h -> fold h into it
    Wo[1] *= h
    bo0, bo1h, bo2, bo3 = (float(bo[0, 0]), float(bo[1, 0]) * h,
                           float(bo[2, 0]), float(bo[3, 0]))

    cS, cV = S0v, V0v    # centering constants for fp16 inputs
    # first-layer bias with t-term and centering folded in: [4, n_steps, 128]
    t_vals = tg[:n_steps].astype(np.float32)
    b1 = (bi[:, None, :] + t_vals[None, :, None] * Wi[:, 0][:, None, :]
          + cS * Wi[:, 1][:, None, :] + cV * Wi[:, 2][:, None, :])
    # device layout: [128 features, n_steps*4] with col t*4+n
    b1_dev = np.ascontiguousarray(b1.transpose(2, 1, 0).reshape(128, n_steps * 4),
                                  np.float32)

    # first-layer weights: rows 32n..32n+1 hold Wi[n, 1:3, :]
    wiT_dev = np.zeros((128, 128), np.float16)
    for n in range(4):
        wiT_dev[32 * n:32 * n + 2, :] = Wi[n, 1:3, :].astype(np.float16)
    whT_dev = np.ascontiguousarray(
        Wh.transpose(2, 0, 1, 3).reshape(128, 12 * 128), np.float16)
    # output weights, zero-padded to M=4 per net: cols 4n..4n+4, col 4n+j
    # is Wo[n] when j == n else 0 (outputs accumulate on psum partitions 0-3)
    woT_dev = np.zeros((128, 16), np.float16)
    for n in range(4):
        woT_dev[:, 4 * n + n] = Wo[n, :, 0].astype(np.float16)
    bh_dev = np.ascontiguousarray(bhv.transpose(2, 0, 1).reshape(128, 12), np.float32)

    strk_dev = np.ascontiguousarray(
        np.tile(np.concatenate([-STRIKES_CALL, -STRIKES_PUT])[None, :], (128, 1)),
        np.float32)

    idx = np.asarray(indices).astype(np.int64).reshape(-1)
    idx_steps = [int((v - 1) % n_steps) for v in idx]
    disc = np.exp(-r * 2.0 * idx.astype(np.float64) / n_steps)

    in_maps = []
    for k in range(N_CORES):
        sl = slice(k * MCL, (k + 1) * MCL)
        in_maps.append({
            "z": np.ascontiguousarray((z[sl, :n_steps] * sqh).T, np.float32),
            "z1": np.ascontiguousarray((z1[sl, :n_steps] * sqh).T, np.float32),
            "wiT": wiT_dev, "whT": whT_dev, "woT": woT_dev,
            "b1": b1_dev, "bh": bh_dev, "strk": strk_dev,
        })
    build_args = dict(idx_steps=idx_steps, c0=c0, bo0=bo0, bo1h=bo1h,
                      bo2=bo2, bo3=bo3, cS=cS, cV=cV, n_steps=n_steps)
    return build_args, in_maps, disc


def _combine(results, disc, idx_steps):
    """Sum per-core [128, 21*n_mat] partials into the [96, 10] output."""
    n_mat = len(idx_steps)
    total = np.zeros((128, 21 * n_mat), np.float64)
    for res in results:
        total += np.asarray(res["acc"], np.float64)
    cols = total.sum(axis=0).reshape(n_mat, 21)
    calls_c = cols[:, 0:10]                  # sum relu(S - Kc)
    calls_p = cols[:, 10:20]                 # sum relu(S - Kp)
    sumS = cols[:, 20:21]                    # sum S
    kc = STRIKES_CALL.astype(np.float64)[None, :]
    kp = STRIKES_PUT.astype(np.float64)[None, :]
    # relu(K - S) = relu(S - K) - S + K  (summed over MC samples)
    puts_c = calls_c - sumS + MC * kc
    puts_p = calls_p - sumS + MC * kp
    out = np.concatenate([calls_c, puts_p, calls_p, puts_c], axis=0) / MC
    out = out * np.concatenate([disc] * 4)[:, None]
    return out.astype(np.float32)


def kernel(**inputs) -> np.ndarray:
    from concourse.bass_utils import run_bass_kernel_spmd
    _install_sync_split()
    build_args, in_maps, disc = _prep_inputs(**inputs)
    nc = build_nc(**build_args)
    res = run_bass_kernel_spmd(nc, in_maps, list(range(N_CORES)))
    return _combine(res.results, disc, build_args["idx_steps"])
